# revision 21
# baseline (speedup 1.0000x reference)
"""PairwiseConv1D — ILV=6 variant: 6-chunk sweeps, banks rotate mod 8.

Same dtype strategy as kernel.py (bf16 in, f32 out, split f32 drains), but
chunks are grouped [6,6,4] per batch with bank = chunk % 8. A bank is
reused 8 chunks later, which with 6-chunk weight sweeps puts every reuse a
constant ~1.54us after the bank frees (vs 1.33us for the tightest bank of
8-chunk sweeps) and caps each drain engine's end-of-sweep chain at 3 ops —
comfortable margins instead of a ~40ns knife edge that risks PE stalls and
p-state resets. Costs 7 extra weight reloads per pass.
"""

import numpy as np
import ml_dtypes

import concourse.bass as bass
import concourse.mybir as mybir
from concourse.bass_utils import run_bass_kernel_spmd

B, L, C, K, F = 32, 8192, 128, 7, 128
NCORES = 8
BPC = B // NCORES
LOUT = L - K + 1  # 8186
CHUNK = 512
NBANK = 8
NCHUNK = 16  # chunks per batch, last = 506
SWEEPS = [(0, 6), (6, 6), (12, 4)]  # (first chunk, n chunks) per batch
SWMAX = 6 * CHUNK  # obuf slot width (cols)
XDMA = 4
XCOLS = L // XDMA
NOBUF = 2

# x-DMA slots each sweep needs (halo K-1 included)
SWEEP_SLOTS = [(0, 1), (2, 3), (3,)]

BF16 = mybir.dt.bfloat16
NPBF16 = ml_dtypes.bfloat16

_nc = None


def _chunk_n(u):
    return CHUNK if u < NCHUNK - 1 else LOUT - (NCHUNK - 1) * CHUNK


# static drain bookkeeping: position parity within a sweep picks the engine
DVE, ACT = 0, 1
_drain_tag = {}  # chunk u -> (engine, per-pass count after this drain)
_sweep_cnt = []  # per sweep index: (dve count after sweep, act count after)
_c = [0, 0]
for _s0, _nsw in SWEEPS:
    for _e in (DVE, ACT):
        for _p in range(_e, _nsw, 2):
            _c[_e] += 1
            _drain_tag[_s0 + _p] = (_e, _c[_e])
    _sweep_cnt.append(tuple(_c))
DRAINS_PER_PASS = _c  # [8, 8]


def _build(reps=1, detect_races=True):
    f32 = mybir.dt.float32
    nc = bass.Bass(detect_race_conditions=detect_races)
    xT = nc.dram_tensor("xT", [BPC, C, L], BF16, kind="ExternalInput")
    w = nc.dram_tensor("w", [K, C, F], BF16, kind="ExternalInput")
    outT = nc.dram_tensor("outT", [BPC, F, LOUT], f32, kind="ExternalOutput")

    G = reps * BPC
    NSW = len(SWEEPS)
    TTS = G * NSW  # total sweeps

    from contextlib import ExitStack

    with ExitStack() as ctx:
        wsb = ctx.enter_context(nc.sbuf_tensor([C, K * F], BF16))
        xbuf0 = ctx.enter_context(nc.sbuf_tensor([C, L], BF16))
        xbuf1 = ctx.enter_context(nc.sbuf_tensor([C, L], BF16))
        obuf = ctx.enter_context(nc.sbuf_tensor([F, NOBUF * SWMAX], f32))
        psum = ctx.enter_context(nc.psum_tensor([F, NBANK * CHUNK], f32))
        wsem = ctx.enter_context(nc.semaphore())
        xsems = [
            ctx.enter_context(nc.semaphore(name=f"xsem{c}")) for c in range(XDMA)
        ]
        pe_sem = ctx.enter_context(nc.semaphore())
        dsems = [
            ctx.enter_context(nc.semaphore(name="dvesem")),
            ctx.enter_context(nc.semaphore(name="actsem")),
        ]
        osems = [
            ctx.enter_context(nc.semaphore(name=f"osem{s}")) for s in range(NOBUF)
        ]
        block = ctx.enter_context(nc.Block())

        xbufs = [xbuf0, xbuf1]

        def _store(sync, Sg):
            g, s = Sg // NSW, Sg % NSW
            b = g % BPC
            s0, nsw = SWEEPS[s]
            cols0 = s0 * CHUNK
            ncols = min(nsw * CHUNK, LOUT - cols0)
            slot = Sg % NOBUF
            dcnt, acnt = _sweep_cnt[s]
            sync.wait_ge(dsems[DVE], DRAINS_PER_PASS[DVE] * g + dcnt)
            sync.wait_ge(dsems[ACT], DRAINS_PER_PASS[ACT] * g + acnt)
            sync.dma_start(
                outT[b, :, cols0 : cols0 + ncols],
                obuf[:, slot * SWMAX : slot * SWMAX + ncols],
            ).then_inc(osems[slot], 16)

        @block.sync
        def _(sync):
            sync.dma_start(
                wsb[:, :], w.ap().rearrange("k c f -> c k f")
            ).then_inc(wsem, 16)
            for g in range(G):
                b = g % BPC
                if g >= 2:
                    sync.wait_ge(pe_sem, NCHUNK * (g - 1))
                xb = xbufs[g % 2]
                for c in range(XDMA):
                    sync.dma_start(
                        xb[:, c * XCOLS : (c + 1) * XCOLS],
                        xT[b, :, c * XCOLS : (c + 1) * XCOLS],
                    ).then_inc(xsems[c], 16)
                if g >= 1:
                    for s in range(NSW):
                        _store(sync, (g - 1) * NSW + s)
            for s in range(NSW):
                _store(sync, (G - 1) * NSW + s)
            for sl in range(NOBUF):
                sync.wait_ge(osems[sl], 16 * (TTS // NOBUF))
            for s in [wsem, pe_sem] + dsems + xsems + osems:
                sync.sem_clear(s)

        @block.tensor
        def _(tensor):
            tensor.wait_ge(wsem, 16)
            xseen = [0] * XDMA
            for g in range(G):
                xb = xbufs[g % 2]
                for s, (s0, nsw) in enumerate(SWEEPS):
                    need = 16 * (g + 1)
                    for c in SWEEP_SLOTS[s]:
                        if xseen[c] < need:
                            tensor.wait_ge(xsems[c], need)
                            xseen[c] = need
                    for k in range(K):
                        for p in range(nsw):
                            u = s0 + p
                            n = _chunk_n(u)
                            bank = u % NBANK
                            if k == 0:
                                # previous user of this bank: chunk u-8 of
                                # this pass, or u+8 of the previous pass
                                if u >= NBANK:
                                    pg, pu = g, u - NBANK
                                else:
                                    pg, pu = g - 1, u + NBANK
                                if pg >= 0:
                                    eng, cnt = _drain_tag[pu]
                                    tensor.wait_ge(
                                        dsems[eng],
                                        DRAINS_PER_PASS[eng] * pg + cnt,
                                    )
                            ins = nc.tensor.matmul(
                                psum[:, bank * CHUNK : bank * CHUNK + n],
                                wsb[:, k * F : (k + 1) * F],
                                xb[:, u * CHUNK + k : u * CHUNK + k + n],
                                start=(k == 0),
                                stop=(k == K - 1),
                                skip_group_check=True,
                            )
                            if k == K - 1:
                                ins.then_inc(pe_sem, 1)

        def drain_body(engine, copy_fn, parity):
            sem = dsems[parity]
            for Sg in range(TTS):
                g, s = Sg // NSW, Sg % NSW
                s0, nsw = SWEEPS[s]
                slot = Sg % NOBUF
                if Sg >= NOBUF:
                    engine.wait_ge(osems[slot], 16 * (Sg // NOBUF))
                for p in range(parity, nsw, 2):
                    u = s0 + p
                    n = _chunk_n(u)
                    bank = u % NBANK
                    engine.wait_ge(pe_sem, NCHUNK * g + u + 1)
                    copy_fn(
                        obuf[:, slot * SWMAX + p * CHUNK :
                             slot * SWMAX + p * CHUNK + n],
                        psum[:, bank * CHUNK : bank * CHUNK + n],
                    ).then_inc(sem, 1)

        @block.vector
        def _(vector):
            drain_body(vector, nc.vector.tensor_copy, DVE)

        @block.scalar
        def _(scalar):
            drain_body(scalar, nc.scalar.copy, ACT)

    return nc


def make_in_maps(x, kernel):
    x = np.asarray(x, dtype=np.float32)
    w = np.ascontiguousarray(np.asarray(kernel, dtype=np.float32)).astype(NPBF16)
    xT = np.ascontiguousarray(
        np.transpose(x[..., 0], (0, 2, 1))
    ).astype(NPBF16)
    return [
        {"xT": xT[i * BPC : (i + 1) * BPC], "w": w} for i in range(NCORES)
    ]


def kernel(x, kernel):
    global _nc
    in_maps = make_in_maps(x, kernel)
    if _nc is None:
        _nc = _build()
    res = run_bass_kernel_spmd(_nc, in_maps, core_ids=list(range(NCORES)))
    outT = np.concatenate(
        [r["outT"].astype(np.float32) for r in res.results], axis=0
    )
    out = np.transpose(outT, (0, 2, 1))[..., None]
    return np.ascontiguousarray(out).astype(np.float32)
